# revision 17
# baseline (speedup 1.0000x reference)
# GCN message-passing kernel for Trainium2 (8 NeuronCores, MPMD).
#
# Math (PyG GCNConv x2 + per-graph MLP readout):
#   A_norm[c,r] = dinv[c] * ew * dinv[r]   (incl. self loops w=1),  dinv = rsqrt(deg)
#   h1 = leaky_relu(A_norm @ x  @ W1 + b1)
#   h2 =            A_norm @ h1 @ W2 + b2
#   z  = reshape(h2, [B, 22*128]);  MLP;  out = tanh(z)*90 + 150
#
# Edges are uniformly random over all nodes (the graphs are NOT closed
# components), so conv2 needs a real device-side gather of h1 rows.
#
# Device plan (2 launches, nodes sharded 22528/core contiguously):
#   host: deg/dinv (weighted in-degree) in numpy; folds dinv[dst]*ew*dinv[src]
#       into selector strip values; builds per-edge x payloads. W2 is folded
#       into L1 (h1w = leaky(h1) @ W2), b2 is folded into the readout bias
#       (bf0_eff = bf0 + tile(b2,22) @ Wf0), so L2 aggregation directly
#       produces h2^T.
#   L1 (MPMD x8): conv1: whole sx/sel resident in SBUF (2 big DMAs);
#       per-chunk matmuls reduce into PSUM per 512-dest group; @W1(fp16)+b1;
#       leaky (fp16); @W2(fp16); PE fp16 transposes; h1w rows fp16, one
#       output DMA.
#   L2 (MPMD x8): conv2: rounds of R=6 dest groups held in PSUM banks; per
#       (round, window) one big dma_gather (int16 idx, fp16 256B rows);
#       per-chunk matmuls accumulate; drain psum -> h2T fp16; readout MLP in
#       fp16; tanh*90+150 -> y.
#
# With PROFILE=True each launch runs under NRT/NTFF profiling and
# LAST_EXEC_NS[name] records neuron-profile exec_time_ns (max over the 8
# concurrently-launched cores).

import numpy as np

N = 180224
E = 1441792
HID = 128
NPG = 22
NCORES = 8
NLOC = N // NCORES          # 22528 nodes per core
B = N // NPG                # 8192 graphs
BLOC = B // NCORES          # 1024 graphs per core
GROUP = 512                 # dest columns per PSUM bank group
P = 128
WIN = 32768                 # int16 gather window (rows)
NWIN = (N + WIN - 1) // WIN
RGRP = 6                    # dest groups per conv2 round (PSUM banks used)


# ----------------------------------------------------------------------------
# host-side structure building
# ----------------------------------------------------------------------------

def _sorted_edges(srcs, dsts, ews):
    order = np.argsort(dsts, kind="stable")
    return srcs[order].astype(np.int64), dsts[order].astype(np.int64), \
        ews[order].astype(np.float32)


def _build_conv1(ss, ds, es, c):
    """Dest-major whole-run packing into full 128-slot chunks."""
    d_loc = ds - c * NLOC
    deg = np.bincount(d_loc, minlength=NLOC)
    run_start = np.concatenate([[0], np.cumsum(deg)])[:-1]

    n_groups = (NLOC + GROUP - 1) // GROUP
    chunk_group, chunk_lo, chunk_span = [], [], []
    run_chunk = np.empty(NLOC, np.int64)
    run_slot = np.empty(NLOC, np.int64)
    groups = [[] for _ in range(n_groups)]
    acc, cur, cur_g = 0, -1, -1
    deg_l = deg.tolist()
    for dl in range(NLOC):
        g = dl // GROUP
        L = deg_l[dl]
        if cur < 0 or g != cur_g or acc + L > 128:
            cur = len(chunk_group)
            chunk_group.append(g)
            chunk_lo.append(dl)
            chunk_span.append(0)
            groups[g].append(cur)
            acc, cur_g = 0, g
        run_chunk[dl] = cur
        run_slot[dl] = cur * 128 + acc
        acc += L
        chunk_span[cur] = dl - chunk_lo[cur] + 1

    n_chunks = len(chunk_group)
    chunk_lo = np.asarray(chunk_lo, np.int64)
    chunk_span = np.asarray(chunk_span, np.int64)
    sel_off = np.concatenate([[0], np.cumsum(chunk_span)])
    S = int(sel_off[-1])

    rank = np.arange(len(ss)) - run_start[d_loc]
    slot = run_slot[d_loc] + rank
    slots_src = np.zeros(n_chunks * 128, np.int64)
    slots_src[slot] = ss
    ch_of_e = run_chunk[d_loc]
    sel_row = (slot % 128).astype(np.int64)
    sel_col = sel_off[ch_of_e] + d_loc - chunk_lo[ch_of_e]

    return dict(slots_src=slots_src, sel_row=sel_row, sel_col=sel_col, S=S,
                n_chunks=n_chunks, chunk_lo=chunk_lo, chunk_span=chunk_span,
                sel_off=sel_off, groups=groups,
                d_loc=d_loc, src=ss, ew=es)


def _build_conv2(ss, ds, es, c, rgrp=RGRP):
    """(round, window, group, dest)-sorted slots; one gather per (round,
    window); chunks never cross a slab column or a group boundary; spans
    accumulate into per-group zeroed PSUM banks (start=False everywhere)."""
    d_loc = ds - c * NLOC
    g_of = d_loc // GROUP
    r_of = g_of // rgrp
    w_of = ss // WIN
    order = np.lexsort((d_loc, g_of, w_of, r_of))
    s2, d2, e2 = ss[order], d_loc[order], es[order]
    n_groups = (NLOC + GROUP - 1) // GROUP
    n_rounds = (n_groups + rgrp - 1) // rgrp

    # (r, w, g) run boundaries
    key = (r_of[order] * NWIN + w_of[order]) * n_groups + g_of[order]
    bnd = np.flatnonzero(np.diff(key)) + 1
    starts = np.concatenate([[0], bnd])
    ends = np.concatenate([bnd, [len(key)]])

    sel_rows, sel_cols, sel_vals = [], [], []
    d_all, s_all = [], []
    batches = []
    all_slots = []
    sel_off = 0
    idx_cols = 0
    cur_rw = None
    bat = None
    slots = None     # per-batch window-relative src ids

    def _close_batch():
        nonlocal idx_cols
        pad = -len(slots) % 32
        slots.extend([0] * pad)
        bat["n"] = len(slots)
        bat["cols"] = (len(slots) + 127) // 128
        idx_cols += len(slots) // 16
        batches.append(bat)
        all_slots.append(np.asarray(slots, np.int64))

    for st, en in zip(starts, ends):
        kk = int(key[st])
        g = kk % n_groups
        rw = kk // n_groups
        r, w = rw // NWIN, rw % NWIN
        if rw != cur_rw:
            if bat is not None:
                _close_batch()
            bat = dict(r=r, w=w, icol=idx_cols, chunks=[])
            cur_rw = rw
            slots = []
        n_raw = en - st
        s_loc = s2[st:en] - w * WIN
        dls = d2[st:en]
        vals = e2[st:en]
        dglob = d2[st:en] + c * NLOC
        sglob = s2[st:en]
        p = 0
        while p < n_raw:
            base_now = len(slots) % 128
            if base_now == 96:
                # PE tiles cannot start at partition 96 -- dead padding
                slots.extend([0] * 32)
                continue
            kcap = {0: 128, 32: 32, 64: 64}[base_now]
            k = min(kcap, n_raw - p)
            lo = int(dls[p:p + k].min())
            hi = int(dls[p:p + k].max())
            span = hi - lo + 1
            bat["chunks"].append(dict(col=len(slots) // 128, base=base_now,
                                      k=int(k), g=g, coff=lo - g * GROUP,
                                      span=span, soff=sel_off))
            sel_rows.append(base_now + np.arange(k))
            sel_cols.append(sel_off + dls[p:p + k] - lo)
            sel_vals.append(vals[p:p + k])
            d_all.append(dglob[p:p + k])
            s_all.append(sglob[p:p + k])
            sel_off += span
            slots.extend(s_loc[p:p + k].tolist())
            p += k
        pad = -len(slots) % 32
        slots.extend([0] * pad)
    if bat is not None:
        _close_batch()
    slot_src = all_slots
    return dict(batches=batches, n_groups=n_groups, n_rounds=n_rounds,
                rgrp=rgrp,
                slot_src=np.concatenate(slot_src),
                sel_row=np.concatenate(sel_rows),
                sel_col=np.concatenate(sel_cols),
                sel_val=np.concatenate(sel_vals),
                d_glob=np.concatenate(d_all),
                s_glob=np.concatenate(s_all),
                S2=int(sel_off), idx_cols=int(idx_cols))


def _conv2_arrays(st, dinv):
    idx = np.zeros((P, st["idx_cols"]), np.int16)
    pos = 0
    for b in st["batches"]:
        n = b["n"]
        blk = st["slot_src"][pos:pos + n].astype(np.int16).reshape(n // 16, 16).T
        idx[:, b["icol"]:b["icol"] + n // 16] = np.tile(blk, (8, 1))
        pos += n
    sel = np.zeros((P, st["S2"]), np.float16)
    vals = st["sel_val"] * dinv[st["d_glob"]] * dinv[st["s_glob"]]
    sel[st["sel_row"], st["sel_col"]] = vals.astype(np.float16)
    return idx, sel


# ----------------------------------------------------------------------------
# device programs
# ----------------------------------------------------------------------------

def _bass_mods():
    import concourse.bass as bass
    import concourse.bacc as bacc
    import concourse.tile as tile
    from concourse import mybir
    return bass, bacc, tile, mybir


def build_l1(core, nloc=None):
    """conv1 + W2 fold: h1w = leaky(agg@W1 + b1) @ W2, fp16 rows out."""
    nloc = NLOC if nloc is None else nloc
    bass, bacc, tile, mybir = _bass_mods()
    from concourse.masks import make_identity
    from contextlib import ExitStack

    n_chunks = core["n_chunks"]
    S = core["S"]
    groups = core["groups"]
    chunk_lo = core["chunk_lo"]
    chunk_span = core["chunk_span"]
    sel_off = core["sel_off"]
    n_groups = len(groups)

    nc = bacc.Bacc("TRN2", target_bir_lowering=False, debug=False, num_devices=1)
    f32 = mybir.dt.float32
    f16 = mybir.dt.float16
    sx = nc.dram_tensor("sx", [P, n_chunks * 3], f16, kind="ExternalInput").ap()
    sel = nc.dram_tensor("sel", [P, S], f16, kind="ExternalInput").ap()
    W1 = nc.dram_tensor("W1", [3, HID], f16, kind="ExternalInput").ap()
    W2 = nc.dram_tensor("W2", [HID, HID], f16, kind="ExternalInput").ap()
    b1 = nc.dram_tensor("b1", [HID, 1], f32, kind="ExternalInput").ap()
    b1s = nc.dram_tensor("b1s", [HID, 1], f32, kind="ExternalInput").ap()
    h1w = nc.dram_tensor("h1w", [nloc, HID], f16, kind="ExternalOutput").ap()

    nt_tot = nloc // P

    with tile.TileContext(nc) as tc, ExitStack() as ctx:
        consts = ctx.enter_context(tc.tile_pool(name="consts", bufs=1))
        big = ctx.enter_context(tc.tile_pool(name="big", bufs=1))
        sb = ctx.enter_context(tc.tile_pool(name="sb", bufs=3))
        psa = ctx.enter_context(tc.tile_pool(name="psa", bufs=2, space="PSUM"))
        psh = ctx.enter_context(tc.tile_pool(name="psh", bufs=2, space="PSUM"))
        pst = ctx.enter_context(tc.tile_pool(name="pst", bufs=2, space="PSUM"))

        W1_t = consts.tile([3, HID], f16)
        nc.sync.dma_start(W1_t[:], W1[:])
        W2_t = consts.tile([HID, HID], f16)
        nc.sync.dma_start(W2_t[:], W2[:])
        b1_t = consts.tile([HID, 1], f32)
        nc.sync.dma_start(b1_t[:], b1[:])
        b1s_t = consts.tile([HID, 1], f32)
        nc.sync.dma_start(b1s_t[:], b1s[:])
        ident = consts.tile([P, P], f16)
        make_identity(nc, ident)

        SX = big.tile([P, n_chunks * 3], f16)
        nc.sync.dma_start(SX[:], sx[:])
        SEL = big.tile([P, S], f16)
        nc.sync.dma_start(SEL[:], sel[:])
        ROWS = big.tile([P, nt_tot, HID], f16)

        def emit_chunks(g):
            agg_ps = psa.tile([3, GROUP], f32, tag="agg",
                              name=f"agg_{g}")
            for j in groups[g]:
                span = int(chunk_span[j])
                coff = int(chunk_lo[j]) - g * GROUP
                soff = int(sel_off[j])
                nc.tensor.matmul(agg_ps[:, coff:coff + span],
                                 lhsT=SX[:, j * 3:j * 3 + 3],
                                 rhs=SEL[:, soff:soff + span],
                                 start=True, stop=True)
            return agg_ps

        def emit_tail(g, agg_ps):
            gwidth = min(GROUP, nloc - g * GROUP)
            agg16 = sb.tile([3, GROUP], f16, tag="agg16", name=f"agg16_{g}")
            nc.vector.tensor_copy(agg16[:, :gwidth], agg_ps[:, :gwidth])

            h1T_ps = psh.tile([HID, GROUP], f32, tag="h1T", name=f"h1T_{g}")
            nc.tensor.matmul(h1T_ps[:, :gwidth], lhsT=W1_t[:],
                             rhs=agg16[:, :gwidth], start=True, stop=True)
            # leaky_relu in fp16: max(x + b1, 0.01 x + 0.01 b1)
            a_t = sb.tile([HID, GROUP], f16, tag="lk_a", name=f"lka_{g}")
            nc.scalar.activation(a_t[:, :gwidth], h1T_ps[:, :gwidth],
                                 mybir.ActivationFunctionType.Identity,
                                 bias=b1_t[:, 0:1], scale=1.0)
            c_t = sb.tile([HID, GROUP], f16, tag="lk_b", name=f"lkb_{g}")
            nc.scalar.activation(c_t[:, :gwidth], h1T_ps[:, :gwidth],
                                 mybir.ActivationFunctionType.Identity,
                                 bias=b1s_t[:, 0:1], scale=0.01)
            hl_t = sb.tile([HID, GROUP], f16, tag="lk_m", name=f"lkm_{g}")
            nc.vector.tensor_tensor(hl_t[:, :gwidth], a_t[:, :gwidth],
                                    c_t[:, :gwidth], op=mybir.AluOpType.max)

            h1w_ps = psh.tile([HID, GROUP], f32, tag="h1w", name=f"h1w_{g}")
            nc.tensor.matmul(h1w_ps[:, :gwidth], lhsT=W2_t[:],
                             rhs=hl_t[:, :gwidth], start=True, stop=True)
            h1w16 = sb.tile([HID, GROUP], f16, tag="h1w16", name=f"h1w16_{g}")
            nc.vector.tensor_copy(h1w16[:, :gwidth], h1w_ps[:, :gwidth])

            nt = (gwidth + P - 1) // P
            for tt in range(nt):
                tr_ps = pst.tile([P, P], f16, tag="tr", name=f"tr_{g}_{tt}")
                nc.tensor.transpose(tr_ps[:], h1w16[:, tt * P:(tt + 1) * P],
                                    ident[:])
                dst = ROWS[:, g * (GROUP // P) + tt, :]
                if tt % 2 == 0:
                    nc.vector.tensor_copy(dst, tr_ps[:])
                else:
                    nc.scalar.activation(dst, tr_ps[:],
                                         mybir.ActivationFunctionType.Identity)

        # software pipeline: chunks(g+1) issue on PE before tail(g), so the
        # PE never waits on group g's ACT/DVE stages
        pend = None
        for g in range(n_groups):
            agg_ps = emit_chunks(g)
            if pend is not None:
                emit_tail(pend[0], pend[1])
            pend = (g, agg_ps)
        emit_tail(pend[0], pend[1])
        out_ap = h1w.rearrange("(t p) f -> p t f", p=P)
        nc.sync.dma_start(out_ap, ROWS[:])
    nc.compile()
    return nc


def build_l1_ct(core, nloc=None, nstrip=4):
    """conv1 with PE column-tiling: chunks round-robin over `nstrip` 32-col
    strips of the array, so LDWEIGHTS/MATMUL fixed costs overlap across
    strips. agg lives at psum partitions {32s..32s+2}; W1 replicated at the
    matching rows of W1xs (host input) sums the strips in the W1 matmul."""
    nloc = NLOC if nloc is None else nloc
    bass, bacc, tile, mybir = _bass_mods()
    from concourse.masks import make_identity
    from contextlib import ExitStack

    n_chunks = core["n_chunks"]
    S = core["S"]
    groups = core["groups"]
    chunk_lo = core["chunk_lo"]
    chunk_span = core["chunk_span"]
    sel_off = core["sel_off"]
    n_groups = len(groups)
    KP = 32 * (nstrip - 1) + 3

    nc = bacc.Bacc("TRN2", target_bir_lowering=False, debug=False, num_devices=1)
    f32 = mybir.dt.float32
    f16 = mybir.dt.float16
    sx = nc.dram_tensor("sx", [P, n_chunks * 3], f16, kind="ExternalInput").ap()
    sel = nc.dram_tensor("sel", [P, S], f16, kind="ExternalInput").ap()
    W1x = nc.dram_tensor("W1x4", [P, HID], f16, kind="ExternalInput").ap()
    W2 = nc.dram_tensor("W2", [HID, HID], f16, kind="ExternalInput").ap()
    b1 = nc.dram_tensor("b1", [HID, 1], f32, kind="ExternalInput").ap()
    b1s = nc.dram_tensor("b1s", [HID, 1], f32, kind="ExternalInput").ap()
    h1w = nc.dram_tensor("h1w", [nloc, HID], f16, kind="ExternalOutput").ap()

    nt_tot = nloc // P

    with tile.TileContext(nc) as tc, ExitStack() as ctx:
        consts = ctx.enter_context(tc.tile_pool(name="consts", bufs=1))
        big = ctx.enter_context(tc.tile_pool(name="big", bufs=1))
        sb = ctx.enter_context(tc.tile_pool(name="sb", bufs=3))
        psa = ctx.enter_context(tc.tile_pool(name="psa", bufs=2, space="PSUM"))
        psh = ctx.enter_context(tc.tile_pool(name="psh", bufs=2, space="PSUM"))
        pst = ctx.enter_context(tc.tile_pool(name="pst", bufs=2, space="PSUM"))

        W1x_t = consts.tile([P, HID], f16)
        nc.sync.dma_start(W1x_t[:], W1x[:])
        W2_t = consts.tile([HID, HID], f16)
        nc.sync.dma_start(W2_t[:], W2[:])
        b1_t = consts.tile([HID, 1], f32)
        nc.sync.dma_start(b1_t[:], b1[:])
        b1s_t = consts.tile([HID, 1], f32)
        nc.sync.dma_start(b1s_t[:], b1s[:])
        ident = consts.tile([P, P], f16)
        make_identity(nc, ident)

        SX = big.tile([P, n_chunks * 3], f16)
        nc.sync.dma_start(SX[:], sx[:])
        SEL = big.tile([P, S], f16)
        nc.sync.dma_start(SEL[:], sel[:])
        ROWS = big.tile([P, nt_tot, HID], f16)

        def emit_chunks(g):
            agg_ps = psa.tile([KP, GROUP], f32, tag="agg", name=f"agg_{g}")
            nc.vector.memset(agg_ps[:], 0.0)
            for jj, j in enumerate(groups[g]):
                s = jj % nstrip
                span = int(chunk_span[j])
                coff = int(chunk_lo[j]) - g * GROUP
                soff = int(sel_off[j])
                nc.tensor.matmul(agg_ps[32 * s:32 * s + 3,
                                        coff:coff + span],
                                 lhsT=SX[:, j * 3:j * 3 + 3],
                                 rhs=SEL[:, soff:soff + span],
                                 start=True, stop=True,
                                 tile_position=(0, 32 * s))
            return agg_ps

        def emit_tail(g, agg_ps):
            gwidth = min(GROUP, nloc - g * GROUP)
            agg16 = sb.tile([KP, GROUP], f16, tag="agg16", name=f"agg16_{g}")
            nc.vector.tensor_copy(agg16[:, :gwidth], agg_ps[:, :gwidth])

            h1T_ps = psh.tile([HID, GROUP], f32, tag="h1T", name=f"h1T_{g}")
            nc.tensor.matmul(h1T_ps[:, :gwidth], lhsT=W1x_t[:KP, :],
                             rhs=agg16[:, :gwidth], start=True, stop=True)
            a_t = sb.tile([HID, GROUP], f16, tag="lk_a", name=f"lka_{g}")
            nc.scalar.activation(a_t[:, :gwidth], h1T_ps[:, :gwidth],
                                 mybir.ActivationFunctionType.Identity,
                                 bias=b1_t[:, 0:1], scale=1.0)
            c_t = sb.tile([HID, GROUP], f16, tag="lk_b", name=f"lkb_{g}")
            nc.scalar.activation(c_t[:, :gwidth], h1T_ps[:, :gwidth],
                                 mybir.ActivationFunctionType.Identity,
                                 bias=b1s_t[:, 0:1], scale=0.01)
            hl_t = sb.tile([HID, GROUP], f16, tag="lk_m", name=f"lkm_{g}")
            nc.vector.tensor_tensor(hl_t[:, :gwidth], a_t[:, :gwidth],
                                    c_t[:, :gwidth], op=mybir.AluOpType.max)

            h1w_ps = psh.tile([HID, GROUP], f32, tag="h1w", name=f"h1w_{g}")
            nc.tensor.matmul(h1w_ps[:, :gwidth], lhsT=W2_t[:],
                             rhs=hl_t[:, :gwidth], start=True, stop=True)
            h1w16 = sb.tile([HID, GROUP], f16, tag="h1w16", name=f"h1w16_{g}")
            nc.vector.tensor_copy(h1w16[:, :gwidth], h1w_ps[:, :gwidth])

            nt = (gwidth + P - 1) // P
            for tt in range(nt):
                tr_ps = pst.tile([P, P], f16, tag="tr", name=f"tr_{g}_{tt}")
                nc.tensor.transpose(tr_ps[:], h1w16[:, tt * P:(tt + 1) * P],
                                    ident[:])
                dst = ROWS[:, g * (GROUP // P) + tt, :]
                if tt % 2 == 0:
                    nc.vector.tensor_copy(dst, tr_ps[:])
                else:
                    nc.scalar.activation(dst, tr_ps[:],
                                         mybir.ActivationFunctionType.Identity)

        pend = None
        for g in range(n_groups):
            agg_ps = emit_chunks(g)
            if pend is not None:
                emit_tail(pend[0], pend[1])
            pend = (g, agg_ps)
        emit_tail(pend[0], pend[1])
        out_ap = h1w.rearrange("(t p) f -> p t f", p=P)
        nc.sync.dma_start(out_ap, ROWS[:])
    nc.compile()
    return nc


def build_l2(st, nloc=None, bloc=None, n_rows=None):
    """conv2 (round-batched fp16 dma_gather + accumulate) + readout MLP."""
    nloc = NLOC if nloc is None else nloc
    bloc = BLOC if bloc is None else bloc
    n_rows = N if n_rows is None else n_rows
    bass, bacc, tile, mybir = _bass_mods()
    from contextlib import ExitStack

    batches = st["batches"]
    n_groups = st["n_groups"]
    n_rounds = st["n_rounds"]
    rgrp = st["rgrp"]
    S2 = st["S2"]
    idx_cols = st["idx_cols"]

    nc = bacc.Bacc("TRN2", target_bir_lowering=False, debug=False,
                   num_devices=1, num_swdge_queues=4)
    f32 = mybir.dt.float32
    f16 = mybir.dt.float16
    h1f = nc.dram_tensor("h1f", [n_rows, HID], f16, kind="ExternalInput").ap()
    idx = nc.dram_tensor("idx", [P, idx_cols], mybir.dt.int16,
                         kind="ExternalInput").ap()
    sel = nc.dram_tensor("sel", [P, S2], f16, kind="ExternalInput").ap()
    Wf0 = nc.dram_tensor("Wf0", [HID, NPG * HID], f16, kind="ExternalInput").ap()
    bf0 = nc.dram_tensor("bf0", [HID, 1], f32, kind="ExternalInput").ap()
    Wf1 = nc.dram_tensor("Wf1", [HID, HID], f16, kind="ExternalInput").ap()
    bf1 = nc.dram_tensor("bf1", [HID, 1], f32, kind="ExternalInput").ap()
    Wout = nc.dram_tensor("Wout", [HID, 1], f16, kind="ExternalInput").ap()
    bo = nc.dram_tensor("bo", [1, 1], f32, kind="ExternalInput").ap()
    y = nc.dram_tensor("y", [bloc], f32, kind="ExternalOutput").ap()

    max_cols = max(b["cols"] for b in batches)
    for b in batches:
        b["sel0"] = b["chunks"][0]["soff"]
        b["selw"] = (b["chunks"][-1]["soff"] + b["chunks"][-1]["span"]
                     - b["sel0"])
    max_bsel = max(b["selw"] for b in batches)

    by_round = [[] for _ in range(n_rounds)]
    for b in batches:
        by_round[b["r"]].append(b)

    with tile.TileContext(nc) as tc, ExitStack() as ctx:
        consts = ctx.enter_context(tc.tile_pool(name="consts", bufs=1))
        big = ctx.enter_context(tc.tile_pool(name="big", bufs=1))
        sb = ctx.enter_context(tc.tile_pool(name="sb", bufs=3))
        idxp = ctx.enter_context(tc.tile_pool(name="idxp", bufs=3))
        slabs = ctx.enter_context(tc.tile_pool(name="slabs", bufs=3))
        ps = ctx.enter_context(tc.tile_pool(name="ps", bufs=1, space="PSUM"))
        pst = ctx.enter_context(tc.tile_pool(name="pst", bufs=2, space="PSUM"))

        Wf0_t = consts.tile([HID, NPG, HID], f16)
        nc.sync.dma_start(Wf0_t[:], Wf0.rearrange("k (j m) -> k j m", j=NPG))
        bf0_t = consts.tile([HID, 1], f32)
        nc.sync.dma_start(bf0_t[:], bf0[:])
        Wf1_t = consts.tile([HID, HID], f16)
        nc.sync.dma_start(Wf1_t[:], Wf1[:])
        bf1_t = consts.tile([HID, 1], f32)
        nc.sync.dma_start(bf1_t[:], bf1[:])
        Wout_t = consts.tile([HID, 1], f16)
        nc.sync.dma_start(Wout_t[:], Wout[:])
        bo_t = consts.tile([1, 1], f32)
        nc.sync.dma_start(bo_t[:], bo[:])
        bf0b_t = consts.tile([HID, 1], f32)
        nc.vector.tensor_scalar_mul(bf0b_t[:], bf0_t[:], 0.01)
        bf1b_t = consts.tile([HID, 1], f32)
        nc.vector.tensor_scalar_mul(bf1b_t[:], bf1_t[:], 0.01)

        h2T = big.tile([HID, nloc], f16)
        qn = 0
        for r in range(n_rounds):
            gs = [g for g in range(r * rgrp, min((r + 1) * rgrp, n_groups))]
            aggs = {}
            for i, g in enumerate(gs):
                aggs[g] = ps.tile([HID, GROUP], f32, tag=f"agg{i}",
                                  name=f"agg_r{r}_{i}")
                nc.vector.memset(aggs[g][:], 0.0)
            for b in by_round[r]:
                n, w, cols = b["n"], b["w"], b["cols"]
                wsz = min(WIN, n_rows - w * WIN)
                s0, sw = b["sel0"], b["selw"]
                sel_t = sb.tile([P, max_bsel], f16, tag="sel")
                nc.sync.dma_start(sel_t[:, :sw], sel[:, s0:s0 + sw])
                idx_t = idxp.tile([P, max_cols * 8], mybir.dt.int16, tag="idx")
                nc.sync.dma_start(idx_t[:, :n // 16],
                                  idx[:, b["icol"]:b["icol"] + n // 16])
                gat_t = slabs.tile([P, max_cols, HID], f16, tag="gat")
                nc.gpsimd.dma_gather(
                    out_ap=gat_t[:, :cols, :],
                    in_ap=h1f[w * WIN:w * WIN + wsz, :],
                    idxs_ap=idx_t[:, :n // 16],
                    num_idxs=n, num_idxs_reg=n, elem_size=HID,
                    single_packet=False, queue_num=qn)
                qn = (qn + 1) % 4
                for ch in b["chunks"]:
                    k, base, col = ch["k"], ch["base"], ch["col"]
                    so = ch["soff"] - s0
                    nc.tensor.matmul(
                        aggs[ch["g"]][:, ch["coff"]:ch["coff"] + ch["span"]],
                        lhsT=gat_t[base:base + k, col, :],
                        rhs=sel_t[base:base + k, so:so + ch["span"]],
                        start=False, stop=True, skip_group_check=True)
            for i, g in enumerate(gs):
                gwidth = min(GROUP, nloc - g * GROUP)
                dst = h2T[:, g * GROUP:g * GROUP + gwidth]
                if i % 2 == 0:
                    nc.scalar.activation(dst, aggs[g][:, :gwidth],
                                         mybir.ActivationFunctionType.Identity)
                else:
                    nc.vector.tensor_copy(dst, aggs[g][:, :gwidth])

        # readout MLP, feature-major, fp16 weights
        GT = 512
        n_gt = (bloc + GT - 1) // GT
        y_sb = big.tile([1, bloc], f32)
        for gt in range(n_gt):
            gw = min(GT, bloc - gt * GT)
            f0_ps = ps.tile([HID, GT], f32, tag="agg0")
            for j in range(NPG):
                zT = h2T[:, gt * GT * NPG + j:
                         gt * GT * NPG + j + (gw - 1) * NPG + 1:NPG]
                nc.tensor.matmul(f0_ps[:, :gw], lhsT=Wf0_t[:, j, :], rhs=zT,
                                 start=(j == 0), stop=(j == NPG - 1))
            a_t = sb.tile([HID, GT], f16, tag="f0a")
            nc.scalar.activation(a_t[:, :gw], f0_ps[:, :gw],
                                 mybir.ActivationFunctionType.Identity,
                                 bias=bf0_t[:, 0:1])
            c_t = sb.tile([HID, GT], f16, tag="f0b")
            nc.scalar.activation(c_t[:, :gw], f0_ps[:, :gw],
                                 mybir.ActivationFunctionType.Identity,
                                 bias=bf0b_t[:, 0:1], scale=0.01)
            f0_t = sb.tile([HID, GT], f16, tag="f0m")
            nc.vector.tensor_tensor(f0_t[:, :gw], a_t[:, :gw], c_t[:, :gw],
                                    op=mybir.AluOpType.max)

            f1_ps = ps.tile([HID, GT], f32, tag="agg1")
            nc.tensor.matmul(f1_ps[:, :gw], lhsT=Wf1_t[:], rhs=f0_t[:, :gw],
                             start=True, stop=True)
            a2_t = sb.tile([HID, GT], f16, tag="f1a")
            nc.scalar.activation(a2_t[:, :gw], f1_ps[:, :gw],
                                 mybir.ActivationFunctionType.Identity,
                                 bias=bf1_t[:, 0:1])
            c2_t = sb.tile([HID, GT], f16, tag="f1b")
            nc.scalar.activation(c2_t[:, :gw], f1_ps[:, :gw],
                                 mybir.ActivationFunctionType.Identity,
                                 bias=bf1b_t[:, 0:1], scale=0.01)
            f1_t = sb.tile([HID, GT], f16, tag="f1m")
            nc.vector.tensor_tensor(f1_t[:, :gw], a2_t[:, :gw], c2_t[:, :gw],
                                    op=mybir.AluOpType.max)

            o_ps = pst.tile([1, GT], f32, tag="o")
            nc.tensor.matmul(o_ps[:, :gw], lhsT=Wout_t[:], rhs=f1_t[:, :gw],
                             start=True, stop=True)
            t_t = sb.tile([1, GT], f32, tag="tanh")
            nc.scalar.activation(t_t[:, :gw], o_ps[:, :gw],
                                 mybir.ActivationFunctionType.Tanh,
                                 bias=bo_t[:, 0:1], scale=1.0)
            nc.vector.tensor_scalar(y_sb[:, gt * GT:gt * GT + gw], t_t[:, :gw],
                                    scalar1=90.0, scalar2=150.0,
                                    op0=mybir.AluOpType.mult,
                                    op1=mybir.AluOpType.add)
        nc.sync.dma_start(y.rearrange("(a b) -> a b", a=1), y_sb[:])
    nc.compile()
    return nc


# ----------------------------------------------------------------------------
# MPMD runner (one program per device, concurrent dispatch)
# ----------------------------------------------------------------------------

def _make_runner(nc, device):
    import jax
    import concourse.mybir as mybir
    from concourse.bass2jax import (install_neuronx_cc_hook, _bass_exec_p,
                                    partition_id_tensor)
    install_neuronx_cc_hook()
    in_names, out_names, out_avals, zero_shapes = [], [], [], []
    part_name = nc.partition_id_tensor.name if nc.partition_id_tensor else None
    for alloc in nc.m.functions[0].allocations:
        if not isinstance(alloc, mybir.MemoryLocationSet):
            continue
        name = alloc.memorylocations[0].name
        if alloc.kind == "ExternalInput":
            if name != part_name:
                in_names.append(name)
        elif alloc.kind == "ExternalOutput":
            out_names.append(name)
            shape = tuple(alloc.tensor_shape)
            dtype = mybir.dt.np(alloc.dtype)
            out_avals.append(jax.core.ShapedArray(shape, dtype))
            zero_shapes.append((shape, dtype))
    n_params = len(in_names)
    all_in = list(in_names) + list(out_names)
    if part_name is not None:
        all_in = all_in + [part_name]
    donate = tuple(range(n_params, n_params + len(out_names)))

    def _body(*args):
        operands = list(args)
        if part_name is not None:
            operands.append(partition_id_tensor())
        outs = _bass_exec_p.bind(
            *operands,
            out_avals=tuple(out_avals),
            in_names=tuple(all_in),
            out_names=tuple(out_names),
            lowering_input_output_aliases=(),
            sim_require_finite=True,
            sim_require_nnan=True,
            nc=nc,
        )
        return tuple(outs)

    jitted = jax.jit(_body, donate_argnums=donate, keep_unused=True)
    return dict(jit=jitted, nc=nc, in_names=in_names, out_names=out_names,
                zero_shapes=zero_shapes, device=device)


# ----------------------------------------------------------------------------
# NTFF profiling (neuron-profile exec_time_ns per launch, PROFILE=True only)
# ----------------------------------------------------------------------------

_AXON_SO = "/opt/axon/libaxon_pjrt.so"


def _profile_hook():
    import ctypes
    lib = ctypes.CDLL(_AXON_SO)
    if not hasattr(lib, "axon_start_nrt_profile"):
        return None
    lib.axon_start_nrt_profile.argtypes = [ctypes.POINTER(ctypes.c_int64),
                                           ctypes.c_size_t]
    lib.axon_start_nrt_profile.restype = ctypes.c_int64
    lib.axon_stop_nrt_profile.argtypes = [ctypes.c_char_p]
    lib.axon_stop_nrt_profile.restype = ctypes.c_int64
    return lib


def _parse_launch_ntffs(tmpdir, runners, name):
    """NTFF -> neuron-profile JSON -> gauge exec_time_ns, per core.

    The axon profile ships one NTFF + NEFF pair per executable; executables
    are numbered in compile order, which matches runner order.
    """
    import glob as _glob
    import os
    import re
    import subprocess

    regex = re.compile(
        r"^(?P<fname>.*)-process(?P<proc>\d{6})-executable(?P<exec>\d{6})"
        r"-device(?P<device>\d{6})-execution-?(?P<execution>\d+).ntff$")
    by_exe = {}
    for f in _glob.glob(os.path.join(tmpdir, "*.ntff")):
        m = regex.match(os.path.basename(f))
        if m:
            exe = int(m.group("exec"))
            key = (int(m.group("execution")), f)
            if exe not in by_exe or key > by_exe[exe]:
                by_exe[exe] = key
    exes = sorted(by_exe)
    exec_ns, traces = {}, {}
    if len(exes) != len(runners):
        print(f"profile[{name}]: expected {len(runners)} ntffs, "
              f"got {len(exes)} -- skipping parse")
        return exec_ns, traces
    from gauge import trn_perfetto
    procs = []
    for core, (r, exe) in enumerate(zip(runners, exes)):
        ntff = by_exe[exe][1]
        neff_path = ntff.split("-device")[0] + ".neff"
        json_path = os.path.join(tmpdir, f"k{core}.json")
        p = subprocess.Popen(
            ["neuron-profile", "view", "--ignore-nc-buf-usage",
             "-s", ntff, "-n", neff_path, "--output-format=json",
             f"--output-file={json_path}", "--ignore-dma-trace"],
            cwd=tmpdir,
            stdout=subprocess.DEVNULL, stderr=subprocess.DEVNULL)
        procs.append((core, r, json_path, p))
    for core, r, json_path, p in procs:
        rc = p.wait()
        if rc != 0 or not os.path.exists(json_path):
            print(f"profile[{name}]: neuron-profile failed for core {core}")
            continue
        insts, trace_path, ens, scopes = trn_perfetto.main(
            json=json_path, kernel_dev_mode=True, bass_kernel=r["nc"].m,
            out_path=os.path.join(tmpdir, f"trace_{name}_core{core}.pftrace"),
            title=f"{name}-core{core}")
        exec_ns[core] = ens
        traces[core] = json_path
    return exec_ns, traces


def _run_mpmd_profiled(name, runners, in_maps):
    import jax
    import tempfile
    lib = _profile_hook()
    handle_args = []
    for r, m in zip(runners, in_maps):
        args = [jax.device_put(np.ascontiguousarray(m[n]), r["device"])
                for n in r["in_names"]]
        args += [jax.device_put(np.zeros(s, d), r["device"])
                 for s, d in r["zero_shapes"]]
        jax.block_until_ready(args)
        comp = r["jit"].lower(*args).compile()
        handle_args.append((comp, args))
    tmpdir = tempfile.mkdtemp(prefix=f"ntff_{name}_")
    dev_ids = [r["device"].id for r in runners]
    import ctypes
    ids = (ctypes.c_int64 * len(dev_ids))(*dev_ids)
    rc = lib.axon_start_nrt_profile(ids, len(dev_ids))
    if rc != 0:
        raise RuntimeError(f"axon_start_nrt_profile rc={rc}")
    try:
        handles = [comp(*args) for comp, args in handle_args]
        jax.block_until_ready(handles)
    finally:
        nfiles = lib.axon_stop_nrt_profile(tmpdir.encode())
        print(f"profile[{name}]: {nfiles} file(s) -> {tmpdir}")
    exec_ns, traces = _parse_launch_ntffs(tmpdir, runners, name)
    LAST_EXEC_NS[name] = max(exec_ns.values()) if exec_ns else None
    LAST_EXEC_PER_CORE[name] = exec_ns
    LAST_TRACES[name] = traces
    return [{n: np.asarray(h[i]) for i, n in enumerate(r["out_names"])}
            for r, h in zip(runners, handles)]


def _run_mpmd(runners, in_maps, name=None):
    import jax
    from concurrent.futures import ThreadPoolExecutor
    if PROFILE and name is not None:
        return _run_mpmd_profiled(name, runners, in_maps)
    handle_args = []
    for r, m in zip(runners, in_maps):
        args = [jax.device_put(np.ascontiguousarray(m[n]), r["device"])
                for n in r["in_names"]]
        args += [jax.device_put(np.zeros(s, d), r["device"])
                 for s, d in r["zero_shapes"]]
        handle_args.append((r, args))
    with ThreadPoolExecutor(max_workers=max(1, len(runners))) as ex:
        handles = list(ex.map(lambda ra: ra[0]["jit"](*ra[1]), handle_args))
    jax.block_until_ready(handles)
    return [{n: np.asarray(h[i]) for i, n in enumerate(r["out_names"])}
            for r, h in zip(runners, handles)]


BENCH = False
PROFILE = False
LAST_TIMINGS = {}
LAST_EXEC_NS = {}
LAST_EXEC_PER_CORE = {}
LAST_TRACES = {}
LAST_H1W = None


def _bench_launch(name, runners, in_maps, iters=3):
    import time as _time
    import jax
    dev_args = []
    for r, m in zip(runners, in_maps):
        dev_args.append([jax.device_put(np.ascontiguousarray(m[n]), r["device"])
                         for n in r["in_names"]])
    best = None
    for _ in range(iters):
        packs = []
        for r, args in zip(runners, dev_args):
            zeros = [jax.device_put(np.zeros(s, d), r["device"])
                     for s, d in r["zero_shapes"]]
            jax.block_until_ready(zeros)
            packs.append((r, args, zeros))
        t0 = _time.perf_counter()
        outs = [r["jit"](*args, *zeros) for r, args, zeros in packs]
        jax.block_until_ready(outs)
        dt = _time.perf_counter() - t0
        best = dt if best is None else min(best, dt)
    LAST_TIMINGS[name] = best


# ----------------------------------------------------------------------------
# host-side input prep (shared with bench scripts)
# ----------------------------------------------------------------------------

def prep_host(x, edge_index, edge_weight, W1, b1, W2, b2,
              Wf0, bf0, Wf1, bf1, Wout, bout):
    x = np.asarray(x, np.float32)
    src = np.asarray(edge_index[0], np.int64)
    dst = np.asarray(edge_index[1], np.int64)
    ew = np.asarray(edge_weight, np.float32)

    loops = np.arange(N, dtype=np.int64)
    srcs = np.concatenate([src, loops])
    dsts = np.concatenate([dst, loops])
    ews = np.concatenate([ew, np.ones(N, np.float32)])
    ss, ds, es = _sorted_edges(srcs, dsts, ews)
    bounds = np.searchsorted(ds, np.arange(NCORES + 1) * NLOC)

    deg = np.bincount(ds, weights=es.astype(np.float64), minlength=N)
    dinv = (1.0 / np.sqrt(deg)).astype(np.float32)

    c1, c2 = [], []
    for c in range(NCORES):
        e0, e1 = bounds[c], bounds[c + 1]
        c1.append(_build_conv1(ss[e0:e1], ds[e0:e1], es[e0:e1], c))
        c2.append(_build_conv2(ss[e0:e1], ds[e0:e1], es[e0:e1], c))

    # L1 inputs
    W1_16 = np.asarray(W1, np.float16)
    W2_16 = np.asarray(W2, np.float16)
    b1_f = np.asarray(b1, np.float32).reshape(HID, 1)
    W1x4 = np.zeros((P, HID), np.float16)
    for s in range(4):
        W1x4[32 * s:32 * s + 3] = W1_16
    l1_ins = []
    for c, st in enumerate(c1):
        vals = st["ew"] * dinv[st["d_loc"] + c * NLOC] * dinv[st["src"]]
        sel = np.zeros((P, st["S"]), np.float16)
        sel[st["sel_row"], st["sel_col"]] = vals.astype(np.float16)
        sx = x[st["slots_src"]].astype(np.float16)
        sx = np.ascontiguousarray(
            sx.reshape(st["n_chunks"], 128, 3).transpose(1, 0, 2)
        ).reshape(P, st["n_chunks"] * 3)
        l1_ins.append(dict(sx=sx, sel=sel, W1=W1_16, W1x4=W1x4, W2=W2_16,
                           b1=b1_f, b1s=b1_f * 0.01))

    # L2 inputs (bf0_eff folds b2; fp16 readout weights)
    Wf0_f = np.asarray(Wf0, np.float64)
    b2_f = np.asarray(b2, np.float64).reshape(HID)
    bf0_eff = (np.asarray(bf0, np.float64).reshape(HID)
               + np.tile(b2_f, NPG) @ Wf0_f).astype(np.float32)
    Wf0_r = np.asarray(Wf0, np.float32).reshape(NPG, HID, HID)
    Wf0_r = np.ascontiguousarray(
        Wf0_r.transpose(1, 0, 2)).reshape(HID, NPG * HID).astype(np.float16)
    l2_common = dict(Wf0=Wf0_r,
                     bf0=bf0_eff.reshape(HID, 1),
                     Wf1=np.asarray(Wf1, np.float16),
                     bf1=np.asarray(bf1, np.float32).reshape(HID, 1),
                     Wout=np.asarray(Wout, np.float16).reshape(HID, 1),
                     bo=np.asarray(bout, np.float32).reshape(1, 1))
    l2_ins = []
    for c, st in enumerate(c2):
        idx_arr, sel2 = _conv2_arrays(st, dinv)
        l2_ins.append(dict(idx=idx_arr, sel=sel2, **l2_common))
    return dict(c1=c1, c2=c2, l1_ins=l1_ins, l2_ins=l2_ins)


# ----------------------------------------------------------------------------
# top-level kernel
# ----------------------------------------------------------------------------

def kernel(x, edge_index, edge_weight, W1, b1, W2, b2,
           Wf0, bf0, Wf1, bf1, Wout, bout):
    import jax

    prep = prep_host(x, edge_index, edge_weight, W1, b1, W2, b2,
                     Wf0, bf0, Wf1, bf1, Wout, bout)
    devices = jax.devices()[:NCORES]

    # ---- L1: conv1 (+W2 fold) ----
    l1_runners = [_make_runner(build_l1(st), devices[c])
                  for c, st in enumerate(prep["c1"])]
    res1 = _run_mpmd(l1_runners, prep["l1_ins"], name="L1")
    h1w_full = np.concatenate([r["h1w"] for r in res1], axis=0)  # fp16
    global LAST_H1W
    LAST_H1W = h1w_full
    if BENCH:
        _bench_launch("L1", l1_runners, prep["l1_ins"])

    # ---- L2: conv2 + readout ----
    l2_runners = [_make_runner(build_l2(st), devices[c])
                  for c, st in enumerate(prep["c2"])]
    l2_ins = [dict(h1f=h1w_full, **m) for m in prep["l2_ins"]]
    res2 = _run_mpmd(l2_runners, l2_ins, name="L2")
    if BENCH:
        _bench_launch("L2", l2_runners, l2_ins)
    y = np.concatenate([r["y"] for r in res2]).reshape(B, 1)
    return y


# revision 26
# speedup vs baseline: 1.2715x; 1.2715x over previous
# GCN message-passing kernel for Trainium2 (8 NeuronCores, MPMD).
#
# Math (PyG GCNConv x2 + per-graph MLP readout):
#   A_norm[c,r] = dinv[c] * ew * dinv[r]   (incl. self loops w=1),  dinv = rsqrt(deg)
#   h1 = leaky_relu(A_norm @ x  @ W1 + b1)
#   h2 =            A_norm @ h1 @ W2 + b2
#   z  = reshape(h2, [B, 22*128]);  MLP;  out = tanh(z)*90 + 150
#
# Edges are uniformly random over all nodes (the graphs are NOT closed
# components), so conv2 needs a real device-side gather of h1 rows.
#
# Device plan (2 launches, nodes sharded 22528/core contiguously):
#   host: deg/dinv (weighted in-degree) in numpy; folds dinv[dst]*ew*dinv[src]
#       into selector strip values; builds per-edge x payloads. W2 is folded
#       into L1 (h1w = leaky(h1) @ W2), b2 is folded into the readout bias
#       (bf0_eff = bf0 + tile(b2,22) @ Wf0), so L2 aggregation directly
#       produces h2^T.
#   L1 (MPMD x8): conv1: whole sx/sel resident in SBUF (2 big DMAs);
#       per-chunk matmuls reduce into PSUM per 512-dest group; @W1(fp16)+b1;
#       leaky (fp16); @W2(fp16); PE fp16 transposes; h1w rows fp16, one
#       output DMA.
#   L2 (MPMD x8): conv2: rounds of R=6 dest groups held in PSUM banks; per
#       (round, window) one big dma_gather (int16 idx, fp16 256B rows);
#       per-chunk matmuls accumulate; drain psum -> h2T fp16; readout MLP in
#       fp16; tanh*90+150 -> y.
#
# With PROFILE=True each launch runs under NRT/NTFF profiling and
# LAST_EXEC_NS[name] records neuron-profile exec_time_ns (max over the 8
# concurrently-launched cores).

import numpy as np

N = 180224
E = 1441792
HID = 128
NPG = 22
NCORES = 8
NLOC = N // NCORES          # 22528 nodes per core
B = N // NPG                # 8192 graphs
BLOC = B // NCORES          # 1024 graphs per core
GROUP = 512                 # dest columns per PSUM bank group
P = 128
WIN = 32768                 # int16 gather window (rows)
NWIN = (N + WIN - 1) // WIN
RGRP = 6                    # dest groups per conv2 round (PSUM banks used)
INTERLEAVE_READOUT = False  # emit readout blocks as their groups drain


# ----------------------------------------------------------------------------
# host-side structure building
# ----------------------------------------------------------------------------

def _sorted_edges(srcs, dsts, ews):
    order = np.argsort(dsts, kind="stable")
    return srcs[order].astype(np.int64), dsts[order].astype(np.int64), \
        ews[order].astype(np.float32)


def _build_conv1(ss, ds, es, c):
    """Dest-major whole-run packing into full 128-slot chunks."""
    d_loc = ds - c * NLOC
    deg = np.bincount(d_loc, minlength=NLOC)
    run_start = np.concatenate([[0], np.cumsum(deg)])[:-1]

    n_groups = (NLOC + GROUP - 1) // GROUP
    chunk_group, chunk_lo, chunk_span = [], [], []
    run_chunk = np.empty(NLOC, np.int64)
    run_slot = np.empty(NLOC, np.int64)
    groups = [[] for _ in range(n_groups)]
    acc, cur, cur_g = 0, -1, -1
    deg_l = deg.tolist()
    for dl in range(NLOC):
        g = dl // GROUP
        L = deg_l[dl]
        if cur < 0 or g != cur_g or acc + L > 128:
            cur = len(chunk_group)
            chunk_group.append(g)
            chunk_lo.append(dl)
            chunk_span.append(0)
            groups[g].append(cur)
            acc, cur_g = 0, g
        run_chunk[dl] = cur
        run_slot[dl] = cur * 128 + acc
        acc += L
        chunk_span[cur] = dl - chunk_lo[cur] + 1

    n_chunks = len(chunk_group)
    chunk_lo = np.asarray(chunk_lo, np.int64)
    chunk_span = np.asarray(chunk_span, np.int64)
    sel_off = np.concatenate([[0], np.cumsum(chunk_span)])
    S = int(sel_off[-1])

    rank = np.arange(len(ss)) - run_start[d_loc]
    slot = run_slot[d_loc] + rank
    slots_src = np.zeros(n_chunks * 128, np.int64)
    slots_src[slot] = ss
    ch_of_e = run_chunk[d_loc]
    sel_row = (slot % 128).astype(np.int64)
    sel_col = sel_off[ch_of_e] + d_loc - chunk_lo[ch_of_e]

    return dict(slots_src=slots_src, sel_row=sel_row, sel_col=sel_col, S=S,
                n_chunks=n_chunks, chunk_lo=chunk_lo, chunk_span=chunk_span,
                sel_off=sel_off, groups=groups,
                d_loc=d_loc, src=ss, ew=es)


def _build_conv2(ss, ds, es, c, rgrp=RGRP):
    """(round, window, group, dest)-sorted slots; one gather per (round,
    window); chunks never cross a slab column or a group boundary; spans
    accumulate into per-group zeroed PSUM banks (start=False everywhere)."""
    d_loc = ds - c * NLOC
    g_of = d_loc // GROUP
    r_of = g_of // rgrp
    w_of = ss // WIN
    order = np.lexsort((d_loc, g_of, w_of, r_of))
    s2, d2, e2 = ss[order], d_loc[order], es[order]
    n_groups = (NLOC + GROUP - 1) // GROUP
    n_rounds = (n_groups + rgrp - 1) // rgrp

    # (r, w, g) run boundaries
    key = (r_of[order] * NWIN + w_of[order]) * n_groups + g_of[order]
    bnd = np.flatnonzero(np.diff(key)) + 1
    starts = np.concatenate([[0], bnd])
    ends = np.concatenate([bnd, [len(key)]])

    sel_rows, sel_cols, sel_vals = [], [], []
    d_all, s_all = [], []
    batches = []
    all_slots = []
    sel_off = 0
    idx_cols = 0
    cur_rw = None
    bat = None
    slots = None     # per-batch window-relative src ids

    def _close_batch():
        nonlocal idx_cols
        pad = -len(slots) % 32
        slots.extend([0] * pad)
        bat["n"] = len(slots)
        bat["cols"] = (len(slots) + 127) // 128
        idx_cols += len(slots) // 16
        batches.append(bat)
        all_slots.append(np.asarray(slots, np.int64))

    for st, en in zip(starts, ends):
        kk = int(key[st])
        g = kk % n_groups
        rw = kk // n_groups
        r, w = rw // NWIN, rw % NWIN
        if rw != cur_rw:
            if bat is not None:
                _close_batch()
            bat = dict(r=r, w=w, icol=idx_cols, chunks=[])
            cur_rw = rw
            slots = []
        n_raw = en - st
        s_loc = s2[st:en] - w * WIN
        dls = d2[st:en]
        vals = e2[st:en]
        dglob = d2[st:en] + c * NLOC
        sglob = s2[st:en]
        p = 0
        while p < n_raw:
            base_now = len(slots) % 128
            if base_now == 96:
                # PE tiles cannot start at partition 96 -- dead padding
                slots.extend([0] * 32)
                continue
            kcap = {0: 128, 32: 32, 64: 64}[base_now]
            k = min(kcap, n_raw - p)
            lo = int(dls[p:p + k].min())
            hi = int(dls[p:p + k].max())
            span = hi - lo + 1
            bat["chunks"].append(dict(col=len(slots) // 128, base=base_now,
                                      k=int(k), g=g, coff=lo - g * GROUP,
                                      span=span, soff=sel_off))
            sel_rows.append(base_now + np.arange(k))
            sel_cols.append(sel_off + dls[p:p + k] - lo)
            sel_vals.append(vals[p:p + k])
            d_all.append(dglob[p:p + k])
            s_all.append(sglob[p:p + k])
            sel_off += span
            slots.extend(s_loc[p:p + k].tolist())
            p += k
        pad = -len(slots) % 32
        slots.extend([0] * pad)
    if bat is not None:
        _close_batch()
    slot_src = all_slots
    return dict(batches=batches, n_groups=n_groups, n_rounds=n_rounds,
                rgrp=rgrp,
                slot_src=np.concatenate(slot_src),
                sel_row=np.concatenate(sel_rows),
                sel_col=np.concatenate(sel_cols),
                sel_val=np.concatenate(sel_vals),
                d_glob=np.concatenate(d_all),
                s_glob=np.concatenate(s_all),
                S2=int(sel_off), idx_cols=int(idx_cols))


def _conv2_arrays(st, dinv):
    idx = np.zeros((P, st["idx_cols"]), np.int16)
    pos = 0
    for b in st["batches"]:
        n = b["n"]
        blk = st["slot_src"][pos:pos + n].astype(np.int16).reshape(n // 16, 16).T
        idx[:, b["icol"]:b["icol"] + n // 16] = np.tile(blk, (8, 1))
        pos += n
    sel = np.zeros((P, st["S2"]), np.float16)
    vals = st["sel_val"] * dinv[st["d_glob"]] * dinv[st["s_glob"]]
    sel[st["sel_row"], st["sel_col"]] = vals.astype(np.float16)
    return idx, sel


# ----------------------------------------------------------------------------
# device programs
# ----------------------------------------------------------------------------

def _bass_mods():
    import concourse.bass as bass
    import concourse.bacc as bacc
    import concourse.tile as tile
    from concourse import mybir
    return bass, bacc, tile, mybir


def build_l1(core, nloc=None):
    """conv1 + W2 fold: h1w = leaky(agg@W1 + b1) @ W2, fp16 rows out."""
    nloc = NLOC if nloc is None else nloc
    bass, bacc, tile, mybir = _bass_mods()
    from concourse.masks import make_identity
    from contextlib import ExitStack

    n_chunks = core["n_chunks"]
    S = core["S"]
    groups = core["groups"]
    chunk_lo = core["chunk_lo"]
    chunk_span = core["chunk_span"]
    sel_off = core["sel_off"]
    n_groups = len(groups)

    nc = bacc.Bacc("TRN2", target_bir_lowering=False, debug=False, num_devices=1)
    f32 = mybir.dt.float32
    f16 = mybir.dt.float16
    sx = nc.dram_tensor("sx", [P, n_chunks * 3], f16, kind="ExternalInput").ap()
    sel = nc.dram_tensor("sel", [P, S], f16, kind="ExternalInput").ap()
    W1 = nc.dram_tensor("W1", [3, HID], f16, kind="ExternalInput").ap()
    W2 = nc.dram_tensor("W2", [HID, HID], f16, kind="ExternalInput").ap()
    b1 = nc.dram_tensor("b1", [HID, 1], f32, kind="ExternalInput").ap()
    b1s = nc.dram_tensor("b1s", [HID, 1], f32, kind="ExternalInput").ap()
    h1w = nc.dram_tensor("h1w", [nloc, HID], f16, kind="ExternalOutput").ap()

    nt_tot = nloc // P

    with tile.TileContext(nc) as tc, ExitStack() as ctx:
        consts = ctx.enter_context(tc.tile_pool(name="consts", bufs=1))
        big = ctx.enter_context(tc.tile_pool(name="big", bufs=1))
        sb = ctx.enter_context(tc.tile_pool(name="sb", bufs=3))
        psa = ctx.enter_context(tc.tile_pool(name="psa", bufs=3, space="PSUM"))
        psh = ctx.enter_context(tc.tile_pool(name="psh", bufs=2, space="PSUM"))
        pst = ctx.enter_context(tc.tile_pool(name="pst", bufs=2, space="PSUM"))

        W1_t = consts.tile([3, HID], f16)
        nc.sync.dma_start(W1_t[:], W1[:])
        W2_t = consts.tile([HID, HID], f16)
        nc.sync.dma_start(W2_t[:], W2[:])
        b1_t = consts.tile([HID, 1], f32)
        nc.sync.dma_start(b1_t[:], b1[:])
        b1s_t = consts.tile([HID, 1], f32)
        nc.sync.dma_start(b1s_t[:], b1s[:])
        ident = consts.tile([P, P], f16)
        make_identity(nc, ident)

        SX = big.tile([P, n_chunks * 3], f16)
        SEL = big.tile([P, S], f16)
        qS = (S + 3) // 4
        qX = (n_chunks * 3 + 3) // 4
        for q in range(4):
            nc.sync.dma_start(SX[:, q * qX:min((q + 1) * qX, n_chunks * 3)],
                              sx[:, q * qX:min((q + 1) * qX, n_chunks * 3)])
            nc.sync.dma_start(SEL[:, q * qS:min((q + 1) * qS, S)],
                              sel[:, q * qS:min((q + 1) * qS, S)])
        ROWS = big.tile([P, nt_tot, HID], f16)

        def emit_chunks(g):
            agg_ps = psa.tile([3, GROUP], f32, tag="agg",
                              name=f"agg_{g}")
            for j in groups[g]:
                span = int(chunk_span[j])
                coff = int(chunk_lo[j]) - g * GROUP
                soff = int(sel_off[j])
                nc.tensor.matmul(agg_ps[:, coff:coff + span],
                                 lhsT=SX[:, j * 3:j * 3 + 3],
                                 rhs=SEL[:, soff:soff + span],
                                 start=True, stop=True)
            return agg_ps

        def emit_tail(g, agg_ps):
            gwidth = min(GROUP, nloc - g * GROUP)
            agg16 = sb.tile([3, GROUP], f16, tag="agg16", name=f"agg16_{g}")
            nc.vector.tensor_copy(agg16[:, :gwidth], agg_ps[:, :gwidth])

            h1T_ps = psh.tile([HID, GROUP], f32, tag="h1T", name=f"h1T_{g}")
            nc.tensor.matmul(h1T_ps[:, :gwidth], lhsT=W1_t[:],
                             rhs=agg16[:, :gwidth], start=True, stop=True)
            # leaky_relu in fp16: max(x + b1, 0.01 x + 0.01 b1)
            a_t = sb.tile([HID, GROUP], f16, tag="lk_a", name=f"lka_{g}")
            nc.scalar.activation(a_t[:, :gwidth], h1T_ps[:, :gwidth],
                                 mybir.ActivationFunctionType.Identity,
                                 bias=b1_t[:, 0:1], scale=1.0)
            c_t = sb.tile([HID, GROUP], f16, tag="lk_b", name=f"lkb_{g}")
            nc.scalar.activation(c_t[:, :gwidth], h1T_ps[:, :gwidth],
                                 mybir.ActivationFunctionType.Identity,
                                 bias=b1s_t[:, 0:1], scale=0.01)
            hl_t = sb.tile([HID, GROUP], f16, tag="lk_m", name=f"lkm_{g}")
            nc.vector.tensor_tensor(hl_t[:, :gwidth], a_t[:, :gwidth],
                                    c_t[:, :gwidth], op=mybir.AluOpType.max)

            h1w_ps = psh.tile([HID, GROUP], f32, tag="h1w", name=f"h1w_{g}")
            nc.tensor.matmul(h1w_ps[:, :gwidth], lhsT=W2_t[:],
                             rhs=hl_t[:, :gwidth], start=True, stop=True)
            h1w16 = sb.tile([HID, GROUP], f16, tag="h1w16", name=f"h1w16_{g}")
            nc.vector.tensor_copy(h1w16[:, :gwidth], h1w_ps[:, :gwidth])

            nt = (gwidth + P - 1) // P
            for tt in range(nt):
                tr_ps = pst.tile([P, P], f16, tag="tr", name=f"tr_{g}_{tt}")
                nc.tensor.transpose(tr_ps[:], h1w16[:, tt * P:(tt + 1) * P],
                                    ident[:])
                dst = ROWS[:, g * (GROUP // P) + tt, :]
                if tt % 2 == 0:
                    nc.vector.tensor_copy(dst, tr_ps[:])
                else:
                    nc.scalar.activation(dst, tr_ps[:],
                                         mybir.ActivationFunctionType.Identity)
            nt0 = g * (GROUP // P)
            oap = h1w.rearrange("(t p) f -> p t f", p=P)
            nc.scalar.dma_start(oap[:, nt0:nt0 + nt, :],
                                ROWS[:, nt0:nt0 + nt, :])

        # software pipeline: chunks(g+1) issue on PE before tail(g), so the
        # PE never waits on group g's ACT/DVE stages
        pend = None
        for g in range(n_groups):
            agg_ps = emit_chunks(g)
            if pend is not None:
                emit_tail(pend[0], pend[1])
            pend = (g, agg_ps)
        emit_tail(pend[0], pend[1])
    nc.compile()
    return nc


def build_l1_ct(core, nloc=None, nstrip=4):
    """conv1 with PE column-tiling: chunks round-robin over `nstrip` 32-col
    strips of the array, so LDWEIGHTS/MATMUL fixed costs overlap across
    strips. agg lives at psum partitions {32s..32s+2}; W1 replicated at the
    matching rows of W1xs (host input) sums the strips in the W1 matmul."""
    nloc = NLOC if nloc is None else nloc
    bass, bacc, tile, mybir = _bass_mods()
    from concourse.masks import make_identity
    from contextlib import ExitStack

    n_chunks = core["n_chunks"]
    S = core["S"]
    groups = core["groups"]
    chunk_lo = core["chunk_lo"]
    chunk_span = core["chunk_span"]
    sel_off = core["sel_off"]
    n_groups = len(groups)
    KP = 32 * (nstrip - 1) + 3

    nc = bacc.Bacc("TRN2", target_bir_lowering=False, debug=False, num_devices=1)
    f32 = mybir.dt.float32
    f16 = mybir.dt.float16
    sx = nc.dram_tensor("sx", [P, n_chunks * 3], f16, kind="ExternalInput").ap()
    sel = nc.dram_tensor("sel", [P, S], f16, kind="ExternalInput").ap()
    W1x = nc.dram_tensor("W1x4", [P, HID], f16, kind="ExternalInput").ap()
    W2 = nc.dram_tensor("W2", [HID, HID], f16, kind="ExternalInput").ap()
    b1 = nc.dram_tensor("b1", [HID, 1], f32, kind="ExternalInput").ap()
    b1s = nc.dram_tensor("b1s", [HID, 1], f32, kind="ExternalInput").ap()
    h1w = nc.dram_tensor("h1w", [nloc, HID], f16, kind="ExternalOutput").ap()

    nt_tot = nloc // P

    with tile.TileContext(nc) as tc, ExitStack() as ctx:
        consts = ctx.enter_context(tc.tile_pool(name="consts", bufs=1))
        big = ctx.enter_context(tc.tile_pool(name="big", bufs=1))
        sb = ctx.enter_context(tc.tile_pool(name="sb", bufs=3))
        psa = ctx.enter_context(tc.tile_pool(name="psa", bufs=3, space="PSUM"))
        psh = ctx.enter_context(tc.tile_pool(name="psh", bufs=2, space="PSUM"))
        pst = ctx.enter_context(tc.tile_pool(name="pst", bufs=2, space="PSUM"))

        W1x_t = consts.tile([P, HID], f16)
        nc.sync.dma_start(W1x_t[:], W1x[:])
        W2_t = consts.tile([HID, HID], f16)
        nc.sync.dma_start(W2_t[:], W2[:])
        b1_t = consts.tile([HID, 1], f32)
        nc.sync.dma_start(b1_t[:], b1[:])
        b1s_t = consts.tile([HID, 1], f32)
        nc.sync.dma_start(b1s_t[:], b1s[:])
        ident = consts.tile([P, P], f16)
        make_identity(nc, ident)

        SX = big.tile([P, n_chunks * 3], f16)
        SEL = big.tile([P, S], f16)
        qS = (S + 3) // 4
        qX = (n_chunks * 3 + 3) // 4
        for q in range(4):
            nc.sync.dma_start(SX[:, q * qX:min((q + 1) * qX, n_chunks * 3)],
                              sx[:, q * qX:min((q + 1) * qX, n_chunks * 3)])
            nc.sync.dma_start(SEL[:, q * qS:min((q + 1) * qS, S)],
                              sel[:, q * qS:min((q + 1) * qS, S)])
        ROWS = big.tile([P, nt_tot, HID], f16)

        def emit_chunks(g):
            agg_ps = psa.tile([KP, GROUP], f32, tag="agg", name=f"agg_{g}")
            nc.vector.memset(agg_ps[:], 0.0)
            for jj, j in enumerate(groups[g]):
                s = jj % nstrip
                span = int(chunk_span[j])
                coff = int(chunk_lo[j]) - g * GROUP
                soff = int(sel_off[j])
                nc.tensor.matmul(agg_ps[32 * s:32 * s + 3,
                                        coff:coff + span],
                                 lhsT=SX[:, j * 3:j * 3 + 3],
                                 rhs=SEL[:, soff:soff + span],
                                 start=True, stop=True,
                                 tile_position=(0, 32 * s))
            return agg_ps

        def emit_tail(g, agg_ps):
            gwidth = min(GROUP, nloc - g * GROUP)
            agg16 = sb.tile([KP, GROUP], f16, tag="agg16", name=f"agg16_{g}")
            nc.vector.tensor_copy(agg16[:, :gwidth], agg_ps[:, :gwidth])

            h1T_ps = psh.tile([HID, GROUP], f32, tag="h1T", name=f"h1T_{g}")
            nc.tensor.matmul(h1T_ps[:, :gwidth], lhsT=W1x_t[:KP, :],
                             rhs=agg16[:, :gwidth], start=True, stop=True)
            hl_t = sb.tile([HID, GROUP], f16, tag="lk_m", name=f"lkm_{g}")
            nc.scalar.activation(hl_t[:, :gwidth], h1T_ps[:, :gwidth],
                                 mybir.ActivationFunctionType.Lrelu,
                                 bias=b1_t[:, 0:1], scale=1.0, alpha=0.01)

            h1w_ps = psh.tile([HID, GROUP], f32, tag="h1w", name=f"h1w_{g}")
            nc.tensor.matmul(h1w_ps[:, :gwidth], lhsT=W2_t[:],
                             rhs=hl_t[:, :gwidth], start=True, stop=True)
            h1w16 = sb.tile([HID, GROUP], f16, tag="h1w16", name=f"h1w16_{g}")
            nc.vector.tensor_copy(h1w16[:, :gwidth], h1w_ps[:, :gwidth])

            nt = (gwidth + P - 1) // P
            for tt in range(nt):
                tr_ps = pst.tile([P, P], f16, tag="tr", name=f"tr_{g}_{tt}")
                nc.tensor.transpose(tr_ps[:], h1w16[:, tt * P:(tt + 1) * P],
                                    ident[:])
                dst = ROWS[:, g * (GROUP // P) + tt, :]
                if tt % 2 == 0:
                    nc.vector.tensor_copy(dst, tr_ps[:])
                else:
                    nc.scalar.activation(dst, tr_ps[:],
                                         mybir.ActivationFunctionType.Identity)
            nt0 = g * (GROUP // P)
            oap = h1w.rearrange("(t p) f -> p t f", p=P)
            nc.scalar.dma_start(oap[:, nt0:nt0 + nt, :],
                                ROWS[:, nt0:nt0 + nt, :])

        pend = None
        for g in range(n_groups):
            agg_ps = emit_chunks(g)
            if pend is not None:
                emit_tail(pend[0], pend[1])
            pend = (g, agg_ps)
        emit_tail(pend[0], pend[1])
    nc.compile()
    return nc


def build_l2(st, nloc=None, bloc=None, n_rows=None):
    """conv2 (round-batched fp16 dma_gather + accumulate) + readout MLP."""
    nloc = NLOC if nloc is None else nloc
    bloc = BLOC if bloc is None else bloc
    n_rows = N if n_rows is None else n_rows
    bass, bacc, tile, mybir = _bass_mods()
    from contextlib import ExitStack

    batches = st["batches"]
    n_groups = st["n_groups"]
    n_rounds = st["n_rounds"]
    rgrp = st["rgrp"]
    S2 = st["S2"]
    idx_cols = st["idx_cols"]

    nc = bacc.Bacc("TRN2", target_bir_lowering=False, debug=False,
                   num_devices=1, num_swdge_queues=4)
    f32 = mybir.dt.float32
    f16 = mybir.dt.float16
    h1f = nc.dram_tensor("h1f", [n_rows, HID], f16, kind="ExternalInput").ap()
    idx = nc.dram_tensor("idx", [P, idx_cols], mybir.dt.int16,
                         kind="ExternalInput").ap()
    sel = nc.dram_tensor("sel", [P, S2], f16, kind="ExternalInput").ap()
    Wf0 = nc.dram_tensor("Wf0", [HID, NPG * HID], f16, kind="ExternalInput").ap()
    bf0 = nc.dram_tensor("bf0", [HID, 1], f32, kind="ExternalInput").ap()
    Wf1 = nc.dram_tensor("Wf1", [HID, HID], f16, kind="ExternalInput").ap()
    bf1 = nc.dram_tensor("bf1", [HID, 1], f32, kind="ExternalInput").ap()
    Wout = nc.dram_tensor("Wout", [HID, 1], f16, kind="ExternalInput").ap()
    bo = nc.dram_tensor("bo", [1, 1], f32, kind="ExternalInput").ap()
    y = nc.dram_tensor("y", [bloc], f32, kind="ExternalOutput").ap()

    max_cols = max(b["cols"] for b in batches)
    for b in batches:
        b["sel0"] = b["chunks"][0]["soff"]
        b["selw"] = (b["chunks"][-1]["soff"] + b["chunks"][-1]["span"]
                     - b["sel0"])
    max_bsel = max(b["selw"] for b in batches)

    by_round = [[] for _ in range(n_rounds)]
    for b in batches:
        by_round[b["r"]].append(b)

    with tile.TileContext(nc) as tc, ExitStack() as ctx:
        consts = ctx.enter_context(tc.tile_pool(name="consts", bufs=1))
        big = ctx.enter_context(tc.tile_pool(name="big", bufs=1))
        sb = ctx.enter_context(tc.tile_pool(name="sb", bufs=4))
        rb = ctx.enter_context(tc.tile_pool(name="rb", bufs=2))
        idxp = ctx.enter_context(tc.tile_pool(name="idxp", bufs=4))
        slabs = ctx.enter_context(tc.tile_pool(name="slabs", bufs=4))
        ps = ctx.enter_context(tc.tile_pool(name="ps", bufs=1, space="PSUM"))

        h2T = big.tile([HID, nloc], f16)
        y_sb = big.tile([1, bloc], f32)
        GT = 512
        n_gt = (bloc + GT - 1) // GT

        state = dict(qn=0, consts=None)

        def emit_round(r):
            gs = [g for g in range(r * rgrp, min((r + 1) * rgrp, n_groups))]
            aggs = {}
            for i, g in enumerate(gs):
                aggs[g] = ps.tile([HID, GROUP], f32, tag=f"agg{i}",
                                  name=f"agg_r{r}_{i}")
                nc.vector.memset(aggs[g][:], 0.0)
            for b in by_round[r]:
                n, w, cols = b["n"], b["w"], b["cols"]
                wsz = min(WIN, n_rows - w * WIN)
                s0, sw = b["sel0"], b["selw"]
                sel_t = sb.tile([P, max_bsel], f16, tag="sel",
                                name=f"sel_{r}_{w}")
                nc.sync.dma_start(sel_t[:, :sw], sel[:, s0:s0 + sw])
                idx_t = idxp.tile([P, max_cols * 8], mybir.dt.int16,
                                  tag="idx", name=f"idx_{r}_{w}")
                nc.sync.dma_start(idx_t[:, :n // 16],
                                  idx[:, b["icol"]:b["icol"] + n // 16])
                gat_t = slabs.tile([P, max_cols, HID], f16, tag="gat",
                                   name=f"gat_{r}_{w}")
                nc.gpsimd.dma_gather(
                    out_ap=gat_t[:, :cols, :],
                    in_ap=h1f[w * WIN:w * WIN + wsz, :],
                    idxs_ap=idx_t[:, :n // 16],
                    num_idxs=n, num_idxs_reg=n, elem_size=HID,
                    single_packet=False, queue_num=state["qn"])
                state["qn"] = (state["qn"] + 1) % 4
                for ch in b["chunks"]:
                    k, base, col = ch["k"], ch["base"], ch["col"]
                    so = ch["soff"] - s0
                    nc.tensor.matmul(
                        aggs[ch["g"]][:, ch["coff"]:ch["coff"] + ch["span"]],
                        lhsT=gat_t[base:base + k, col, :],
                        rhs=sel_t[base:base + k, so:so + ch["span"]],
                        start=False, stop=True, skip_group_check=True)
            for i, g in enumerate(gs):
                gwidth = min(GROUP, nloc - g * GROUP)
                dst = h2T[:, g * GROUP:g * GROUP + gwidth]
                if i % 2 == 0:
                    nc.scalar.activation(dst, aggs[g][:, :gwidth],
                                         mybir.ActivationFunctionType.Identity)
                else:
                    nc.vector.tensor_copy(dst, aggs[g][:, :gwidth])

        def emit_consts():
            Wf0_t = consts.tile([HID, NPG, HID], f16)
            nc.sync.dma_start(Wf0_t[:],
                              Wf0.rearrange("k (j m) -> k j m", j=NPG))
            bf0_t = consts.tile([HID, 1], f32)
            nc.sync.dma_start(bf0_t[:], bf0[:])
            Wf1_t = consts.tile([HID, HID], f16)
            nc.sync.dma_start(Wf1_t[:], Wf1[:])
            bf1_t = consts.tile([HID, 1], f32)
            nc.sync.dma_start(bf1_t[:], bf1[:])
            Wout_t = consts.tile([HID, 1], f16)
            nc.sync.dma_start(Wout_t[:], Wout[:])
            bo_t = consts.tile([1, 1], f32)
            nc.sync.dma_start(bo_t[:], bo[:])
            bf0b_t = consts.tile([HID, 1], f32)
            nc.vector.tensor_scalar_mul(bf0b_t[:], bf0_t[:], 0.01)
            bf1b_t = consts.tile([HID, 1], f32)
            nc.vector.tensor_scalar_mul(bf1b_t[:], bf1_t[:], 0.01)
            state["consts"] = (Wf0_t, bf0_t, Wf1_t, bf1_t, Wout_t, bo_t,
                               bf0b_t, bf1b_t)

        def emit_readout(gt):
            (Wf0_t, bf0_t, Wf1_t, bf1_t, Wout_t, bo_t,
             bf0b_t, bf1b_t) = state["consts"]
            gw = min(GT, bloc - gt * GT)
            f0_ps = ps.tile([HID, GT], f32, tag="f0", name=f"f0_{gt}")
            for j in range(NPG):
                zT = h2T[:, gt * GT * NPG + j:
                         gt * GT * NPG + j + (gw - 1) * NPG + 1:NPG]
                nc.tensor.matmul(f0_ps[:, :gw], lhsT=Wf0_t[:, j, :], rhs=zT,
                                 start=(j == 0), stop=(j == NPG - 1))
            f0_t = rb.tile([HID, GT], f16, tag="f0m", name=f"f0m_{gt}")
            nc.scalar.activation(f0_t[:, :gw], f0_ps[:, :gw],
                                 mybir.ActivationFunctionType.Lrelu,
                                 bias=bf0_t[:, 0:1], alpha=0.01)

            f1_ps = ps.tile([HID, GT], f32, tag="f1", name=f"f1_{gt}")
            nc.tensor.matmul(f1_ps[:, :gw], lhsT=Wf1_t[:], rhs=f0_t[:, :gw],
                             start=True, stop=True)
            f1_t = rb.tile([HID, GT], f16, tag="f1m", name=f"f1m_{gt}")
            nc.scalar.activation(f1_t[:, :gw], f1_ps[:, :gw],
                                 mybir.ActivationFunctionType.Lrelu,
                                 bias=bf1_t[:, 0:1], alpha=0.01)

            o_ps = ps.tile([HID, GT], f32, tag="f0", name=f"o_{gt}")
            nc.tensor.matmul(o_ps[0:1, :gw], lhsT=Wout_t[:],
                             rhs=f1_t[:, :gw], start=True, stop=True)
            t_t = rb.tile([1, GT], f32, tag="tanh", name=f"t_{gt}")
            nc.scalar.activation(t_t[:, :gw], o_ps[0:1, :gw],
                                 mybir.ActivationFunctionType.Tanh,
                                 bias=bo_t[:, 0:1], scale=1.0)
            nc.vector.tensor_scalar(y_sb[:, gt * GT:gt * GT + gw], t_t[:, :gw],
                                    scalar1=90.0, scalar2=150.0,
                                    op0=mybir.AluOpType.mult,
                                    op1=mybir.AluOpType.add)

        # INTERLEAVE_READOUT: emit each readout block as soon as its
        # groups have drained (saves the serial tail) -- bisect knob.
        emit_consts()
        if INTERLEAVE_READOUT:
            ro_after = {}
            for gt in range(n_gt):
                need_g = min(NPG * (gt + 1), n_groups) - 1
                ro_after[(need_g // rgrp)] = gt
            for r in range(n_rounds):
                emit_round(r)
                if r in ro_after:
                    emit_readout(ro_after[r])
        else:
            for r in range(n_rounds):
                emit_round(r)
            for gt in range(n_gt):
                emit_readout(gt)
        nc.sync.dma_start(y.rearrange("(a b) -> a b", a=1), y_sb[:])
    nc.compile()
    return nc


# ----------------------------------------------------------------------------
# MPMD runner (one program per device, concurrent dispatch)
# ----------------------------------------------------------------------------

def _make_runner(nc, device):
    import jax
    import concourse.mybir as mybir
    from concourse.bass2jax import (install_neuronx_cc_hook, _bass_exec_p,
                                    partition_id_tensor)
    install_neuronx_cc_hook()
    in_names, out_names, out_avals, zero_shapes = [], [], [], []
    part_name = nc.partition_id_tensor.name if nc.partition_id_tensor else None
    for alloc in nc.m.functions[0].allocations:
        if not isinstance(alloc, mybir.MemoryLocationSet):
            continue
        name = alloc.memorylocations[0].name
        if alloc.kind == "ExternalInput":
            if name != part_name:
                in_names.append(name)
        elif alloc.kind == "ExternalOutput":
            out_names.append(name)
            shape = tuple(alloc.tensor_shape)
            dtype = mybir.dt.np(alloc.dtype)
            out_avals.append(jax.core.ShapedArray(shape, dtype))
            zero_shapes.append((shape, dtype))
    n_params = len(in_names)
    all_in = list(in_names) + list(out_names)
    if part_name is not None:
        all_in = all_in + [part_name]
    donate = tuple(range(n_params, n_params + len(out_names)))

    def _body(*args):
        operands = list(args)
        if part_name is not None:
            operands.append(partition_id_tensor())
        outs = _bass_exec_p.bind(
            *operands,
            out_avals=tuple(out_avals),
            in_names=tuple(all_in),
            out_names=tuple(out_names),
            lowering_input_output_aliases=(),
            sim_require_finite=True,
            sim_require_nnan=True,
            nc=nc,
        )
        return tuple(outs)

    jitted = jax.jit(_body, donate_argnums=donate, keep_unused=True)
    return dict(jit=jitted, nc=nc, in_names=in_names, out_names=out_names,
                zero_shapes=zero_shapes, device=device)


# ----------------------------------------------------------------------------
# NTFF profiling (neuron-profile exec_time_ns per launch, PROFILE=True only)
# ----------------------------------------------------------------------------

_AXON_SO = "/opt/axon/libaxon_pjrt.so"


def _profile_hook():
    import ctypes
    lib = ctypes.CDLL(_AXON_SO)
    if not hasattr(lib, "axon_start_nrt_profile"):
        return None
    lib.axon_start_nrt_profile.argtypes = [ctypes.POINTER(ctypes.c_int64),
                                           ctypes.c_size_t]
    lib.axon_start_nrt_profile.restype = ctypes.c_int64
    lib.axon_stop_nrt_profile.argtypes = [ctypes.c_char_p]
    lib.axon_stop_nrt_profile.restype = ctypes.c_int64
    return lib


def _parse_launch_ntffs(tmpdir, runners, name):
    """NTFF -> neuron-profile JSON -> gauge exec_time_ns, per core.

    The axon profile ships one NTFF + NEFF pair per executable; executables
    are numbered in compile order, which matches runner order.
    """
    import glob as _glob
    import os
    import re
    import subprocess

    regex = re.compile(
        r"^(?P<fname>.*)-process(?P<proc>\d{6})-executable(?P<exec>\d{6})"
        r"-device(?P<device>\d{6})-execution-?(?P<execution>\d+).ntff$")
    by_exe = {}
    for f in _glob.glob(os.path.join(tmpdir, "*.ntff")):
        m = regex.match(os.path.basename(f))
        if m:
            exe = int(m.group("exec"))
            key = (int(m.group("execution")), f)
            if exe not in by_exe or key > by_exe[exe]:
                by_exe[exe] = key
    exes = sorted(by_exe)
    exec_ns, traces = {}, {}
    if len(exes) != len(runners):
        print(f"profile[{name}]: expected {len(runners)} ntffs, "
              f"got {len(exes)} -- skipping parse")
        return exec_ns, traces
    from gauge import trn_perfetto
    procs = []
    for core, (r, exe) in enumerate(zip(runners, exes)):
        ntff = by_exe[exe][1]
        neff_path = ntff.split("-device")[0] + ".neff"
        json_path = os.path.join(tmpdir, f"k{core}.json")
        p = subprocess.Popen(
            ["neuron-profile", "view", "--ignore-nc-buf-usage",
             "-s", ntff, "-n", neff_path, "--output-format=json",
             f"--output-file={json_path}", "--ignore-dma-trace"],
            cwd=tmpdir,
            stdout=subprocess.DEVNULL, stderr=subprocess.DEVNULL)
        procs.append((core, r, json_path, p))
    for core, r, json_path, p in procs:
        rc = p.wait()
        if rc != 0 or not os.path.exists(json_path):
            print(f"profile[{name}]: neuron-profile failed for core {core}")
            continue
        insts, trace_path, ens, scopes = trn_perfetto.main(
            json=json_path, kernel_dev_mode=True, bass_kernel=r["nc"].m,
            out_path=os.path.join(tmpdir, f"trace_{name}_core{core}.pftrace"),
            title=f"{name}-core{core}")
        exec_ns[core] = ens
        traces[core] = json_path
    return exec_ns, traces


def _run_mpmd_profiled(name, runners, in_maps):
    import jax
    import tempfile
    lib = _profile_hook()
    handle_args = []
    for r, m in zip(runners, in_maps):
        args = [jax.device_put(np.ascontiguousarray(m[n]), r["device"])
                for n in r["in_names"]]
        args += [jax.device_put(np.zeros(s, d), r["device"])
                 for s, d in r["zero_shapes"]]
        jax.block_until_ready(args)
        comp = r["jit"].lower(*args).compile()
        handle_args.append((comp, args))
    tmpdir = tempfile.mkdtemp(prefix=f"ntff_{name}_")
    dev_ids = [r["device"].id for r in runners]
    import ctypes
    ids = (ctypes.c_int64 * len(dev_ids))(*dev_ids)
    rc = lib.axon_start_nrt_profile(ids, len(dev_ids))
    if rc != 0:
        raise RuntimeError(f"axon_start_nrt_profile rc={rc}")
    try:
        handles = [comp(*args) for comp, args in handle_args]
        jax.block_until_ready(handles)
    finally:
        nfiles = lib.axon_stop_nrt_profile(tmpdir.encode())
        print(f"profile[{name}]: {nfiles} file(s) -> {tmpdir}")
    exec_ns, traces = _parse_launch_ntffs(tmpdir, runners, name)
    LAST_EXEC_NS[name] = max(exec_ns.values()) if exec_ns else None
    LAST_EXEC_PER_CORE[name] = exec_ns
    LAST_TRACES[name] = traces
    return [{n: np.asarray(h[i]) for i, n in enumerate(r["out_names"])}
            for r, h in zip(runners, handles)]


def _run_mpmd(runners, in_maps, name=None):
    import jax
    from concurrent.futures import ThreadPoolExecutor
    if PROFILE and name is not None:
        return _run_mpmd_profiled(name, runners, in_maps)
    handle_args = []
    for r, m in zip(runners, in_maps):
        args = [jax.device_put(np.ascontiguousarray(m[n]), r["device"])
                for n in r["in_names"]]
        args += [jax.device_put(np.zeros(s, d), r["device"])
                 for s, d in r["zero_shapes"]]
        handle_args.append((r, args))
    with ThreadPoolExecutor(max_workers=max(1, len(runners))) as ex:
        handles = list(ex.map(lambda ra: ra[0]["jit"](*ra[1]), handle_args))
    jax.block_until_ready(handles)
    return [{n: np.asarray(h[i]) for i, n in enumerate(r["out_names"])}
            for r, h in zip(runners, handles)]


BENCH = False
PROFILE = False
LAST_TIMINGS = {}
LAST_EXEC_NS = {}
LAST_EXEC_PER_CORE = {}
LAST_TRACES = {}
LAST_H1W = None


def _bench_launch(name, runners, in_maps, iters=3):
    import time as _time
    import jax
    dev_args = []
    for r, m in zip(runners, in_maps):
        dev_args.append([jax.device_put(np.ascontiguousarray(m[n]), r["device"])
                         for n in r["in_names"]])
    best = None
    for _ in range(iters):
        packs = []
        for r, args in zip(runners, dev_args):
            zeros = [jax.device_put(np.zeros(s, d), r["device"])
                     for s, d in r["zero_shapes"]]
            jax.block_until_ready(zeros)
            packs.append((r, args, zeros))
        t0 = _time.perf_counter()
        outs = [r["jit"](*args, *zeros) for r, args, zeros in packs]
        jax.block_until_ready(outs)
        dt = _time.perf_counter() - t0
        best = dt if best is None else min(best, dt)
    LAST_TIMINGS[name] = best


# ----------------------------------------------------------------------------
# host-side input prep (shared with bench scripts)
# ----------------------------------------------------------------------------

def prep_host(x, edge_index, edge_weight, W1, b1, W2, b2,
              Wf0, bf0, Wf1, bf1, Wout, bout):
    x = np.asarray(x, np.float32)
    src = np.asarray(edge_index[0], np.int64)
    dst = np.asarray(edge_index[1], np.int64)
    ew = np.asarray(edge_weight, np.float32)

    loops = np.arange(N, dtype=np.int64)
    srcs = np.concatenate([src, loops])
    dsts = np.concatenate([dst, loops])
    ews = np.concatenate([ew, np.ones(N, np.float32)])
    ss, ds, es = _sorted_edges(srcs, dsts, ews)
    bounds = np.searchsorted(ds, np.arange(NCORES + 1) * NLOC)

    deg = np.bincount(ds, weights=es.astype(np.float64), minlength=N)
    dinv = (1.0 / np.sqrt(deg)).astype(np.float32)

    c1, c2 = [], []
    for c in range(NCORES):
        e0, e1 = bounds[c], bounds[c + 1]
        c1.append(_build_conv1(ss[e0:e1], ds[e0:e1], es[e0:e1], c))
        c2.append(_build_conv2(ss[e0:e1], ds[e0:e1], es[e0:e1], c))

    # L1 inputs
    W1_16 = np.asarray(W1, np.float16)
    W2_16 = np.asarray(W2, np.float16)
    b1_f = np.asarray(b1, np.float32).reshape(HID, 1)
    W1x4 = np.zeros((P, HID), np.float16)
    for s in range(4):
        W1x4[32 * s:32 * s + 3] = W1_16
    l1_ins = []
    for c, st in enumerate(c1):
        vals = st["ew"] * dinv[st["d_loc"] + c * NLOC] * dinv[st["src"]]
        sel = np.zeros((P, st["S"]), np.float16)
        sel[st["sel_row"], st["sel_col"]] = vals.astype(np.float16)
        sx = x[st["slots_src"]].astype(np.float16)
        sx = np.ascontiguousarray(
            sx.reshape(st["n_chunks"], 128, 3).transpose(1, 0, 2)
        ).reshape(P, st["n_chunks"] * 3)
        l1_ins.append(dict(sx=sx, sel=sel, W1=W1_16, W1x4=W1x4, W2=W2_16,
                           b1=b1_f, b1s=b1_f * 0.01))

    # L2 inputs (bf0_eff folds b2; fp16 readout weights)
    Wf0_f = np.asarray(Wf0, np.float64)
    b2_f = np.asarray(b2, np.float64).reshape(HID)
    bf0_eff = (np.asarray(bf0, np.float64).reshape(HID)
               + np.tile(b2_f, NPG) @ Wf0_f).astype(np.float32)
    Wf0_r = np.asarray(Wf0, np.float32).reshape(NPG, HID, HID)
    Wf0_r = np.ascontiguousarray(
        Wf0_r.transpose(1, 0, 2)).reshape(HID, NPG * HID).astype(np.float16)
    l2_common = dict(Wf0=Wf0_r,
                     bf0=bf0_eff.reshape(HID, 1),
                     Wf1=np.asarray(Wf1, np.float16),
                     bf1=np.asarray(bf1, np.float32).reshape(HID, 1),
                     Wout=np.asarray(Wout, np.float16).reshape(HID, 1),
                     bo=np.asarray(bout, np.float32).reshape(1, 1))
    l2_ins = []
    for c, st in enumerate(c2):
        idx_arr, sel2 = _conv2_arrays(st, dinv)
        l2_ins.append(dict(idx=idx_arr, sel=sel2, **l2_common))
    return dict(c1=c1, c2=c2, l1_ins=l1_ins, l2_ins=l2_ins)


# ----------------------------------------------------------------------------
# top-level kernel
# ----------------------------------------------------------------------------

def kernel(x, edge_index, edge_weight, W1, b1, W2, b2,
           Wf0, bf0, Wf1, bf1, Wout, bout):
    import jax

    prep = prep_host(x, edge_index, edge_weight, W1, b1, W2, b2,
                     Wf0, bf0, Wf1, bf1, Wout, bout)
    devices = jax.devices()[:NCORES]

    # ---- L1: conv1 (+W2 fold) ----
    l1_runners = [_make_runner(build_l1_ct(st), devices[c])
                  for c, st in enumerate(prep["c1"])]
    res1 = _run_mpmd(l1_runners, prep["l1_ins"], name="L1")
    h1w_full = np.concatenate([r["h1w"] for r in res1], axis=0)  # fp16
    global LAST_H1W
    LAST_H1W = h1w_full
    if BENCH:
        _bench_launch("L1", l1_runners, prep["l1_ins"])

    # ---- L2: conv2 + readout ----
    l2_runners = [_make_runner(build_l2(st), devices[c])
                  for c, st in enumerate(prep["c2"])]
    l2_ins = [dict(h1f=h1w_full, **m) for m in prep["l2_ins"]]
    res2 = _run_mpmd(l2_runners, l2_ins, name="L2")
    if BENCH:
        _bench_launch("L2", l2_runners, l2_ins)
    y = np.concatenate([r["y"] for r in res2]).reshape(B, 1)
    return y
